# revision 26
# baseline (speedup 1.0000x reference)
"""Trainium2 Bass kernel for nn_Length_Regulator (DurationPredictor + LR expansion).

Sharding: pure data-parallel over batch (32 samples -> 4 per core x 8 cores),
params replicated. Per-core program:
  - DurationPredictor: 4 conv1d(k=3, SAME) layers as PE matmuls (float32r,
    D-major layout), 2 LayerNorms (stats via ones-matmul over partitions),
    final linear head -> dur_preds.
  - LR expansion: host computes searchsorted frame->phoneme indices (the
    reference itself computes T host-side); device gathers rows with
    dma_gather (SWDGE) from a [4*512+1, 384] table whose last row is zero
    (used for padding frames), stores [T_p, 384] per sample. Runs on the
    DMA engines fully overlapped with the conv matmuls on PE.
"""

import sys

for _p in ("/opt/trn_rl_repo", "/root/.axon_site/_ro/trn_rl_repo"):
    if _p not in sys.path:
        sys.path.append(_p)

import numpy as np

import concourse.bass as bass
import concourse.bacc as bacc
import concourse.mybir as mybir
import concourse.tile as tile
import bass_rust
from concourse.vector_clock import ScopedClock
from concourse.bass_utils import run_bass_kernel_spmd
from concourse.masks import make_identity
from concourse import library_config

F32 = mybir.dt.float32
F32R = mybir.dt.float32r
I16 = mybir.dt.int16
AF = mybir.ActivationFunctionType

B, L, D = 32, 512, 384
NCORES = 8
BS = B // NCORES          # samples per core
DT = D // 128             # d tiles (3)
LN_EPS = 1e-5
GMAX = 5                  # gather chunk: columns of 128 frames per dma_gather

# ---------------------------------------------------------------------------
# Tile framework patch: this container's walrus rejects >1 sem-wait on the
# kernel-tail InstDrain ("Too many sync wait commands"). Split the global-clock
# waits onto one nop per wait instead.
_PATCHED = False


def _patch_tile_drain():
    global _PATCHED
    if _PATCHED:
        return
    _PATCHED = True

    def _drain_and_barrier(self, tick_clock, wait_clock):
        drain_inst = self.nc.sync.drain()
        wait_clock.add_sem_waits(
            drain_inst.ins, ScopedClock({None: tick_clock.global_clock})
        )
        si = drain_inst.ins.sync_info
        waits = list(si.on_wait) if si is not None else []
        if len(waits) > 1:
            si.on_wait = waits[:1]
            for k in range(1, len(waits)):
                extra = self.nc.sync.nop(nofuse=True, hint="tile_drain_wait_split")
                extra.ins.sync_info = bass_rust.SyncInfo(
                    on_wait=[waits[k]], on_update=[]
                )
        self.nc.all_engine_barrier()
        popped = self.nc._tile_sem_poison_stack.pop()
        assert popped is self._sem_poison
        self.nc.clear_and_free_semaphores(list(self.sems.allocated().values()))
        self.nc.all_engine_barrier()

    tile.TileContext._drain_and_barrier = _drain_and_barrier


# ---------------------------------------------------------------------------


def _build_program(T_p):
    """One SPMD program; every core runs it on its own 4-sample shard."""
    _patch_tile_drain()
    C = T_p // 128            # gather columns per sample
    CH = C // 16 * 2          # int16 idx cols per sample... (see idx packing)
    IDXC = T_p // 16          # wrapped idx columns per sample

    nc = bacc.Bacc("TRN2", target_bir_lowering=False, debug=False, num_devices=NCORES)
    xz = nc.declare_dram_parameter("xz", [BS * L + 1, D], F32, isOutput=False)
    wts = nc.declare_dram_parameter("wts", [4, 128, 27 * 128], F32, isOutput=False)
    cvb = nc.declare_dram_parameter("cvb", [4, D], F32, isOutput=False)
    lng = nc.declare_dram_parameter("lng", [2, D], F32, isOutput=False)
    lnb = nc.declare_dram_parameter("lnb", [2, D], F32, isOutput=False)
    linw = nc.declare_dram_parameter("linw", [D], F32, isOutput=False)
    linb = nc.declare_dram_parameter("linb", [1, 1], F32, isOutput=False)
    idxd = nc.declare_dram_parameter("idx", [BS, 128, IDXC], I16, isOutput=False)
    outd = nc.declare_dram_parameter("out", [BS, T_p, D], F32, isOutput=True)
    durd = nc.declare_dram_parameter("dur", [BS, L], F32, isOutput=True)

    with tile.TileContext(nc) as tc:
        with (
            tc.tile_pool(name="const", bufs=1) as const,
            tc.tile_pool(name="wpool", bufs=1) as wpool,
            tc.tile_pool(name="acts", bufs=3) as acts,
            tc.tile_pool(name="stats", bufs=2) as stats,
            tc.tile_pool(name="gather", bufs=4) as gather,
            tc.tile_pool(name="io", bufs=4) as io,
            tc.tile_pool(name="ppool", bufs=1, space="PSUM") as ppool,
        ):
            # ---- constants (weights first: conv1a blocks on layer 0) ----
            nc.gpsimd.load_library(library_config.mlp)
            ident = const.tile([128, 128], F32)
            make_identity(nc, ident)
            ones_f = const.tile([128, 128], F32)
            nc.gpsimd.memset(ones_f, 1.0 / D)
            ones = const.tile([128, 128], F32R)
            nc.vector.tensor_copy(ones, ones_f)
            eps_sb = const.tile([128, 1], F32)
            nc.gpsimd.memset(eps_sb, LN_EPS)
            w_sb = [[const.tile([128, 9 * 128], F32R, name=f"w_sb_{_l}_{_o}")
                     for _o in range(DT)] for _l in range(4)]
            for _l in range(4):
                for _o in range(DT):
                    nc.gpsimd.dma_start(
                        out=w_sb[_l][_o], in_=wts[_l, :, _o * 9 * 128:(_o + 1) * 9 * 128]
                    )
            cvb_sb = const.tile([128, 4, DT], F32)
            lng_sb = const.tile([128, 2, DT], F32)
            lnb_sb = const.tile([128, 2, DT], F32)
            linw_sb = const.tile([128, DT], F32R)
            linb_sb = const.tile([1, 1], F32)
            idx_sb = const.tile([128, BS, IDXC], I16)

            def emit_param_loads():
                nc.sync.dma_start(out=lng_sb, in_=lng.rearrange("l (i p) -> p l i", p=128))
                nc.sync.dma_start(out=lnb_sb, in_=lnb.rearrange("l (i p) -> p l i", p=128))
                nc.gpsimd.dma_start(out=linw_sb, in_=linw.rearrange("(i p) -> p i", p=128))
                nc.sync.dma_start(out=linb_sb, in_=linb[:, :])
                nc.sync.dma_start(out=idx_sb, in_=idxd.rearrange("s p c -> p s c"))

            def w_tile(layer, o, i, tap):
                blk = i * 3 + tap
                return w_sb[layer][o][:, blk * 128:(blk + 1) * 128]

            def emit_gather(s):
                c0 = 0
                while c0 < C:
                    g = min(GMAX, C - c0)
                    gt = gather.tile([128, GMAX, D], F32, tag="gt", name=f"gt_{s}_{c0}")
                    nc.gpsimd.dma_gather(
                        out_ap=gt[:, :g, :],
                        in_ap=xz[:, :],
                        idxs_ap=idx_sb[:, s, c0 * 8:(c0 + g) * 8],
                        num_idxs=128 * g,
                        num_idxs_reg=128 * g,
                        elem_size=D,
                    )
                    dst = outd[s, c0 * 128:(c0 + g) * 128, :].rearrange(
                        "(c p) d -> p c d", p=128
                    )
                    nc.sync.dma_start(out=dst, in_=gt[:, :g, :])
                    c0 += g

            def emit_transpose(s):
                h0 = [acts.tile([128, L + 2], F32R, name=f"h0_{i}_{s}", tag=f"h0_{i}", bufs=2) for i in range(DT)]
                pxt = [
                    ppool.tile([128, L], F32, name=f"pxt_{i}_{s}", tag=f"pxt_{i}", bufs=1, space="PSUM")
                    for i in range(DT)
                ]
                for c in range(4):
                    xn = io.tile([128, D], F32, tag="xn", name=f"xn_{s}_{c}")
                    nc.sync.dma_start(
                        out=xn, in_=xz[s * L + c * 128:s * L + (c + 1) * 128, :]
                    )
                    for i in range(DT):
                        nc.tensor.transpose(
                            pxt[i][:, c * 128:(c + 1) * 128],
                            xn[:, i * 128:(i + 1) * 128],
                            ident,
                        )
                for i in range(DT):
                    nc.vector.memset(h0[i][:, 0:1].bitcast(F32), 0.0)
                    nc.vector.memset(h0[i][:, L + 1:L + 2].bitcast(F32), 0.0)
                    nc.vector.tensor_copy(h0[i][:, 1:L + 1], pxt[i])
                return h0

            def conv_layer(s, layer, hin, tag, stats_cb=None):
                # stats_cb(i, hout_i): emit per-tile stats work lagged by one chunk
                hout = [
                    acts.tile([128, L + 2], F32R, name=f"{tag}{layer}_{i}_{s}", tag=f"{tag}_{i}", bufs=2) for i in range(DT)
                ]
                for o in range(DT):
                    pc = ppool.tile([128, L], F32, tag="pconv", bufs=2, space="PSUM", name=f"pc_{s}_{layer}_{o}")
                    first = True
                    for tap in (1, 0, 2):
                        for i in range(DT):
                            nc.tensor.matmul(
                                pc[:, :],
                                w_tile(layer, o, i, tap),
                                hin[i][:, tap:tap + L],
                                start=first,
                                stop=(tap == 2 and i == DT - 1),
                            )
                            first = False
                    nc.vector.memset(hout[o][:, 0:1].bitcast(F32), 0.0)
                    nc.vector.memset(hout[o][:, L + 1:L + 2].bitcast(F32), 0.0)
                    nc.scalar.activation(
                        out=hout[o][:, 1:L + 1],
                        in_=pc,
                        func=AF.Relu,
                        bias=cvb_sb[:, layer, o:o + 1],
                        scale=1.0,
                    )
                    if stats_cb is not None and o >= 1:
                        stats_cb(o - 1, hout[o - 1])
                if stats_cb is not None:
                    stats_cb(DT - 1, hout[DT - 1])
                return hout

            def make_stats(s, ln):
                """Returns (stats_cb, finish) where stats_cb emits per-tile sum/
                sumsq matmuls and finish() emits the scalar chain -> (mu, rstd)."""
                psum = ppool.tile([128, L], F32, tag="pstat", bufs=1, space="PSUM", name=f"pstat_{s}_{ln}")
                psq = ppool.tile([128, L], F32, tag="psq", bufs=2, space="PSUM", name=f"psq_{s}_{ln}")
                sqs = [acts.tile([128, L], F32R, name=f"sq_{i}_{s}_{ln}", tag="sq", bufs=4) for i in range(DT)]

                done = {}

                def stats_cb(i, hti):
                    nc.vector.tensor_mul(sqs[i], hti[:, 1:L + 1].bitcast(F32), hti[:, 1:L + 1].bitcast(F32))
                    nc.tensor.matmul(
                        psum[:, :], ones[:, :], hti[:, 1:L + 1],
                        start=(i == 0), stop=(i == DT - 1),
                    )
                    nc.tensor.matmul(
                        psq[:, :], ones[:, :], sqs[i][:, :],
                        start=(i == 0), stop=(i == DT - 1),
                    )
                    if i == DT - 1:
                        # emit the scalar chain right away: frees the stats
                        # PSUM banks quickly and finishes early on DVE/ACT
                        mu = stats.tile([128, L], F32, tag="mu", bufs=2, name=f"mu_{s}_{ln}")
                        nc.scalar.copy(out=mu, in_=psum)
                        musq = stats.tile([128, L], F32, tag="musq", bufs=2, name=f"musq_{s}_{ln}")
                        nc.vector.tensor_mul(musq, mu, mu)
                        var = stats.tile([128, L], F32, tag="var", bufs=2, name=f"var_{s}_{ln}")
                        nc.vector.tensor_sub(var, psq, musq)
                        std = stats.tile([128, L], F32, tag="std", bufs=2, name=f"std_{s}_{ln}")
                        nc.scalar.activation(
                            out=std, in_=var, func=AF.Sqrt, bias=eps_sb[:, 0:1], scale=1.0
                        )
                        rstd = stats.tile([128, L], F32, tag="rstd", bufs=2, name=f"rstd_{s}_{ln}")
                        nc.vector.reciprocal(out=rstd, in_=std)
                        done["mu"] = mu
                        done["rstd"] = rstd

                def finish():
                    return done["mu"], done["rstd"]

                return stats_cb, finish

            def emit_ln_apply(s, ln, hin, mu, rstd, tag):
                hout = [
                    acts.tile([128, L + 2], F32R, name=f"{tag}ln{ln}_{i}_{s}", tag=f"{tag}_{i}", bufs=3) for i in range(DT)
                ]
                for i in range(DT):
                    t = acts.tile([128, L], F32, tag="lnt", bufs=3, name=f"lnt_{s}_{ln}_{i}")
                    nc.vector.tensor_sub(t, hin[i][:, 1:L + 1].bitcast(F32), mu)
                    nc.vector.tensor_mul(t, t, rstd)
                    nc.vector.memset(hout[i][:, 0:1].bitcast(F32), 0.0)
                    nc.vector.memset(hout[i][:, L + 1:L + 2].bitcast(F32), 0.0)
                    nc.scalar.activation(
                        out=hout[i][:, 1:L + 1],
                        in_=t,
                        func=AF.Identity,
                        bias=lnb_sb[:, ln, i:i + 1],
                        scale=lng_sb[:, ln, i:i + 1],
                    )
                return hout

            def emit_linear(s, h6):
                pl = ppool.tile([1, L], F32, tag="psq", bufs=2, space="PSUM", name=f"pl_{s}")
                for i in range(DT):
                    nc.tensor.matmul(
                        pl[:, :],
                        linw_sb[:, i:i + 1],
                        h6[i][:, 1:L + 1],
                        start=(i == 0),
                        stop=(i == DT - 1),
                    )
                dur_sb = io.tile([1, L], F32, tag="dur", name=f"dur_{s}")
                nc.scalar.activation(
                    out=dur_sb, in_=pl, func=AF.Identity, bias=linb_sb[0:1, 0:1],
                    scale=1.0,
                )
                nc.sync.dma_start(out=durd[s:s + 1, :], in_=dur_sb)

            # ---- software-pipelined emission over samples ----
            stA = {}   # s -> (finishLN1, h2)
            stB = {}   # s -> (finishLN2, h5)

            def phase_A(s, h0=None):
                if h0 is None:
                    h0 = emit_transpose(s)
                h1 = conv_layer(s, 0, h0, "ha")
                cb1, fin1 = make_stats(s, 0)
                h2 = conv_layer(s, 1, h1, "hb", stats_cb=cb1)
                stA[s] = (fin1, h2)

            def phase_B(s):
                emit_gather(s)
                fin1, h2 = stA.pop(s)
                mu1, rstd1 = fin1()
                h3 = emit_ln_apply(s, 0, h2, mu1, rstd1, "hc")
                h4 = conv_layer(s, 2, h3, "ha")
                cb2, fin2 = make_stats(s, 1)
                h5 = conv_layer(s, 3, h4, "hb", stats_cb=cb2)
                stB[s] = (fin2, h5)

            stC = {}

            def phase_Cchain(s):
                fin2, h5 = stB.pop(s)
                mu2, rstd2 = fin2()
                h6 = emit_ln_apply(s, 1, h5, mu2, rstd2, "hd")
                stC[s] = h6

            def phase_Clin(s):
                emit_linear(s, stC.pop(s))

            h0_first = emit_transpose(0)
            nc.sync.dma_start(out=cvb_sb, in_=cvb.rearrange("l (i p) -> p l i", p=128))
            phase_A(0, h0_first)
            emit_param_loads()
            phase_A(1)
            phase_B(0)
            phase_A(2)
            phase_B(1)
            phase_Cchain(0)
            phase_A(3)
            phase_B(2)
            phase_Cchain(1)
            phase_Clin(0)
            phase_Cchain(2)
            phase_B(3)
            phase_Cchain(3)
            phase_Clin(1)
            phase_Clin(2)
            phase_Clin(3)

    nc.compile()
    return nc


_prog_cache = {}


def _get_program(T_p):
    if T_p not in _prog_cache:
        _prog_cache[T_p] = _build_program(T_p)
    return _prog_cache[T_p]


def kernel(x, durations, c1a_w, c1a_b, c1b_w, c1b_b, ln1_g, ln1_b,
           c2a_w, c2a_b, c2b_w, c2b_b, ln2_g, ln2_b, lin_w, lin_b):
    x = np.ascontiguousarray(np.asarray(x, dtype=np.float32))
    durations = np.asarray(durations, dtype=np.int32)

    # ---- host-side index computation (mirrors reference LR exactly) ----
    cum = np.cumsum(durations, axis=1)              # [B, L]
    total = cum[:, -1]                              # [B]
    T = int(total.max())
    T_p = max(128, -(-T // 128) * 128)
    C = T_p // 128

    t_ar = np.arange(T_p)
    ZROW = BS * L                                    # zero row index in xz
    # per sample: frame t -> row (s*L + searchsorted(cum, t, 'right')) or ZROW
    idx_all = np.empty((B, T_p), dtype=np.int16)
    for b in range(B):
        ii = np.searchsorted(cum[b], t_ar, side="right")
        valid = t_ar < total[b]
        loc = np.where(valid, (b % BS) * L + np.minimum(ii, L - 1), ZROW)
        idx_all[b] = loc.astype(np.int16)

    # ---- pack per-core inputs ----
    # weights, pre-transposed to [d_in, d_out] blocks, layer-major
    def pack_w(w):
        # w: [D_out, D_in, 3] -> [128, 27*128]; block b = o*9 + i*3 + tap
        out = np.empty((128, 27 * 128), dtype=np.float32)
        for o in range(DT):
            for i in range(DT):
                for tap in range(3):
                    blk = o * 9 + i * 3 + tap
                    out[:, blk * 128:(blk + 1) * 128] = (
                        w[o * 128:(o + 1) * 128, i * 128:(i + 1) * 128, tap].T
                    )
        return out

    wts = np.stack([pack_w(np.asarray(w, np.float32))
                    for w in (c1a_w, c1b_w, c2a_w, c2b_w)])   # [4,128,3456]
    cvb = np.stack([np.asarray(v, np.float32)
                    for v in (c1a_b, c1b_b, c2a_b, c2b_b)])   # [4,384]
    lng = np.stack([np.asarray(ln1_g, np.float32), np.asarray(ln2_g, np.float32)])
    lnb = np.stack([np.asarray(ln1_b, np.float32), np.asarray(ln2_b, np.float32)])
    linw = np.asarray(lin_w, np.float32).reshape(D)
    linb = np.asarray(lin_b, np.float32).reshape(1, 1)

    in_maps = []
    for core in range(NCORES):
        sl = slice(core * BS, (core + 1) * BS)
        xs = x[sl].reshape(BS * L, D)
        xzc = np.concatenate([xs, np.zeros((1, D), np.float32)], axis=0)
        idxc = idx_all[sl]                              # [BS, T_p]
        # wrap: element j -> [j % 16, j // 16]
        idxw = np.ascontiguousarray(
            idxc.reshape(BS, T_p // 16, 16).transpose(0, 2, 1)
        )                                               # [BS, 16, T_p//16]
        idxw = np.ascontiguousarray(np.tile(idxw, (1, 8, 1)))  # [BS, 128, .]
        in_maps.append({
            "xz": xzc, "wts": wts, "cvb": cvb, "lng": lng, "lnb": lnb,
            "linw": linw, "linb": linb, "idx": idxw,
        })

    nc = _get_program(T_p)
    res = run_bass_kernel_spmd(nc, in_maps, list(range(NCORES)))

    out_full = np.empty((B, T, D), dtype=np.float32)
    dur_full = np.empty((B, L), dtype=np.float32)
    for core in range(NCORES):
        r = res.results[core]
        out_full[core * BS:(core + 1) * BS] = r["out"][:, :T, :]
        dur_full[core * BS:(core + 1) * BS] = r["dur"]
    return (out_full, dur_full)


# revision 27
# speedup vs baseline: 1.0801x; 1.0801x over previous
"""Trainium2 Bass kernel for nn_Length_Regulator (DurationPredictor + LR expansion).

Sharding: pure data-parallel over batch (32 samples -> 4 per core x 8 cores),
params replicated. Per-core program:
  - DurationPredictor: 4 conv1d(k=3, SAME) layers as PE matmuls (float32r,
    D-major layout), 2 LayerNorms (stats via ones-matmul over partitions),
    final linear head -> dur_preds.
  - LR expansion: host computes searchsorted frame->phoneme indices (the
    reference itself computes T host-side); device gathers rows with
    dma_gather (SWDGE) from a [4*512+1, 384] table whose last row is zero
    (used for padding frames), stores [T_p, 384] per sample. Runs on the
    DMA engines fully overlapped with the conv matmuls on PE.
"""

import sys

for _p in ("/opt/trn_rl_repo", "/root/.axon_site/_ro/trn_rl_repo"):
    if _p not in sys.path:
        sys.path.append(_p)

import numpy as np

import concourse.bass as bass
import concourse.bacc as bacc
import concourse.mybir as mybir
import concourse.tile as tile
import bass_rust
from concourse.vector_clock import ScopedClock
from concourse.bass_utils import run_bass_kernel_spmd
from concourse.masks import make_identity
from concourse import library_config

F32 = mybir.dt.float32
F32R = mybir.dt.float32r
I16 = mybir.dt.int16
AF = mybir.ActivationFunctionType

B, L, D = 32, 512, 384
NCORES = 8
BS = B // NCORES          # samples per core
DT = D // 128             # d tiles (3)
LN_EPS = 1e-5
GMAX = 5                  # gather chunk: columns of 128 frames per dma_gather

# ---------------------------------------------------------------------------
# Tile framework patch: this container's walrus rejects >1 sem-wait on the
# kernel-tail InstDrain ("Too many sync wait commands"). Split the global-clock
# waits onto one nop per wait instead.
_PATCHED = False


def _patch_tile_drain():
    global _PATCHED
    if _PATCHED:
        return
    _PATCHED = True

    def _drain_and_barrier(self, tick_clock, wait_clock):
        drain_inst = self.nc.sync.drain()
        wait_clock.add_sem_waits(
            drain_inst.ins, ScopedClock({None: tick_clock.global_clock})
        )
        si = drain_inst.ins.sync_info
        waits = list(si.on_wait) if si is not None else []
        if len(waits) > 1:
            si.on_wait = waits[:1]
            for k in range(1, len(waits)):
                extra = self.nc.sync.nop(nofuse=True, hint="tile_drain_wait_split")
                extra.ins.sync_info = bass_rust.SyncInfo(
                    on_wait=[waits[k]], on_update=[]
                )
        self.nc.all_engine_barrier()
        popped = self.nc._tile_sem_poison_stack.pop()
        assert popped is self._sem_poison
        self.nc.clear_and_free_semaphores(list(self.sems.allocated().values()))
        self.nc.all_engine_barrier()

    tile.TileContext._drain_and_barrier = _drain_and_barrier


# ---------------------------------------------------------------------------


def _build_program(T_p):
    """One SPMD program; every core runs it on its own 4-sample shard."""
    _patch_tile_drain()
    C = T_p // 128            # gather columns per sample
    CH = C // 16 * 2          # int16 idx cols per sample... (see idx packing)
    IDXC = T_p // 16          # wrapped idx columns per sample

    nc = bacc.Bacc("TRN2", target_bir_lowering=False, debug=False, num_devices=NCORES)
    xz = nc.declare_dram_parameter("xz", [BS * L + 1, D], F32, isOutput=False)
    wts = nc.declare_dram_parameter("wts", [4, 128, 27 * 128], F32, isOutput=False)
    cvb = nc.declare_dram_parameter("cvb", [4, D], F32, isOutput=False)
    lng = nc.declare_dram_parameter("lng", [2, D], F32, isOutput=False)
    lnb = nc.declare_dram_parameter("lnb", [2, D], F32, isOutput=False)
    linw = nc.declare_dram_parameter("linw", [D], F32, isOutput=False)
    linb = nc.declare_dram_parameter("linb", [1, 1], F32, isOutput=False)
    idxd = nc.declare_dram_parameter("idx", [BS, 128, IDXC], I16, isOutput=False)
    outd = nc.declare_dram_parameter("out", [BS, T_p, D], F32, isOutput=True)
    durd = nc.declare_dram_parameter("dur", [BS, L], F32, isOutput=True)

    with tile.TileContext(nc) as tc:
        with (
            tc.tile_pool(name="const", bufs=1) as const,
            tc.tile_pool(name="wpool", bufs=1) as wpool,
            tc.tile_pool(name="acts", bufs=3) as acts,
            tc.tile_pool(name="stats", bufs=2) as stats,
            tc.tile_pool(name="gather", bufs=4) as gather,
            tc.tile_pool(name="io", bufs=4) as io,
            tc.tile_pool(name="ppool", bufs=1, space="PSUM") as ppool,
        ):
            # ---- constants (weights first: conv1a blocks on layer 0) ----
            nc.gpsimd.load_library(library_config.mlp)
            ident = const.tile([128, 128], F32)
            make_identity(nc, ident)
            ones_f = const.tile([128, 128], F32)
            nc.gpsimd.memset(ones_f, 1.0 / D)
            ones = const.tile([128, 128], F32R)
            nc.vector.tensor_copy(ones, ones_f)
            eps_sb = const.tile([128, 1], F32)
            nc.gpsimd.memset(eps_sb, LN_EPS)
            w_sb = [[const.tile([128, 9 * 128], F32R, name=f"w_sb_{_l}_{_o}")
                     for _o in range(DT)] for _l in range(4)]
            for _l in range(4):
                for _o in range(DT):
                    nc.gpsimd.dma_start(
                        out=w_sb[_l][_o], in_=wts[_l, :, _o * 9 * 128:(_o + 1) * 9 * 128]
                    )
            cvb_sb = const.tile([128, 4, DT], F32)
            lng_sb = const.tile([128, 2, DT], F32)
            lnb_sb = const.tile([128, 2, DT], F32)
            linw_sb = const.tile([128, DT], F32R)
            linb_sb = const.tile([1, 1], F32)
            idx_sb = const.tile([128, BS, IDXC], I16)

            def emit_param_loads():
                nc.sync.dma_start(out=lng_sb, in_=lng.rearrange("l (i p) -> p l i", p=128))
                nc.sync.dma_start(out=lnb_sb, in_=lnb.rearrange("l (i p) -> p l i", p=128))
                nc.gpsimd.dma_start(out=linw_sb, in_=linw.rearrange("(i p) -> p i", p=128))
                nc.sync.dma_start(out=linb_sb, in_=linb[:, :])
                nc.sync.dma_start(out=idx_sb, in_=idxd.rearrange("s p c -> p s c"))

            def w_tile(layer, o, i, tap):
                blk = i * 3 + tap
                return w_sb[layer][o][:, blk * 128:(blk + 1) * 128]

            def emit_gather(s):
                c0 = 0
                while c0 < C:
                    g = min(GMAX, C - c0)
                    gt = gather.tile([128, GMAX, D], F32, tag="gt", name=f"gt_{s}_{c0}")
                    nc.gpsimd.dma_gather(
                        out_ap=gt[:, :g, :],
                        in_ap=xz[:, :],
                        idxs_ap=idx_sb[:, s, c0 * 8:(c0 + g) * 8],
                        num_idxs=128 * g,
                        num_idxs_reg=128 * g,
                        elem_size=D,
                    )
                    dst = outd[s, c0 * 128:(c0 + g) * 128, :].rearrange(
                        "(c p) d -> p c d", p=128
                    )
                    nc.sync.dma_start(out=dst, in_=gt[:, :g, :])
                    c0 += g

            def emit_transpose(s):
                h0 = [acts.tile([128, L + 2], F32R, name=f"h0_{i}_{s}", tag=f"h0_{i}", bufs=2) for i in range(DT)]
                pxt = [
                    ppool.tile([128, L], F32, name=f"pxt_{i}_{s}", tag=f"pxt_{i}", bufs=1, space="PSUM")
                    for i in range(DT)
                ]
                for c in range(4):
                    xn = io.tile([128, D], F32, tag="xn", name=f"xn_{s}_{c}")
                    nc.sync.dma_start(
                        out=xn, in_=xz[s * L + c * 128:s * L + (c + 1) * 128, :]
                    )
                    for i in range(DT):
                        nc.tensor.transpose(
                            pxt[i][:, c * 128:(c + 1) * 128],
                            xn[:, i * 128:(i + 1) * 128],
                            ident,
                        )
                for i in range(DT):
                    nc.vector.memset(h0[i][:, 0:1].bitcast(F32), 0.0)
                    nc.vector.memset(h0[i][:, L + 1:L + 2].bitcast(F32), 0.0)
                    nc.vector.tensor_copy(h0[i][:, 1:L + 1], pxt[i])
                return h0

            def conv_layer(s, layer, hin, tag, stats_cb=None):
                # stats_cb(i, hout_i): emit per-tile stats work lagged by one chunk
                hout = [
                    acts.tile([128, L + 2], F32R, name=f"{tag}{layer}_{i}_{s}", tag=f"{tag}_{i}", bufs=2) for i in range(DT)
                ]
                for o in range(DT):
                    pc = ppool.tile([128, L], F32, tag="pconv", bufs=2, space="PSUM", name=f"pc_{s}_{layer}_{o}")
                    first = True
                    for tap in (1, 0, 2):
                        for i in range(DT):
                            nc.tensor.matmul(
                                pc[:, :],
                                w_tile(layer, o, i, tap),
                                hin[i][:, tap:tap + L],
                                start=first,
                                stop=(tap == 2 and i == DT - 1),
                            )
                            first = False
                    nc.vector.memset(hout[o][:, 0:1].bitcast(F32), 0.0)
                    nc.vector.memset(hout[o][:, L + 1:L + 2].bitcast(F32), 0.0)
                    nc.scalar.activation(
                        out=hout[o][:, 1:L + 1],
                        in_=pc,
                        func=AF.Relu,
                        bias=cvb_sb[:, layer, o:o + 1],
                        scale=1.0,
                    )
                    if stats_cb is not None and o >= 1:
                        stats_cb(o - 1, hout[o - 1])
                if stats_cb is not None:
                    stats_cb(DT - 1, hout[DT - 1])
                return hout

            def make_stats(s, ln):
                """Returns (stats_cb, finish) where stats_cb emits per-tile sum/
                sumsq matmuls and finish() emits the scalar chain -> (mu, rstd)."""
                psum = ppool.tile([128, L], F32, tag="pstat", bufs=1, space="PSUM", name=f"pstat_{s}_{ln}")
                psq = ppool.tile([128, L], F32, tag="psq", bufs=2, space="PSUM", name=f"psq_{s}_{ln}")
                sqs = [acts.tile([128, L], F32R, name=f"sq_{i}_{s}_{ln}", tag="sq", bufs=4) for i in range(DT)]

                done = {}

                def stats_cb(i, hti):
                    nc.vector.tensor_mul(sqs[i], hti[:, 1:L + 1].bitcast(F32), hti[:, 1:L + 1].bitcast(F32))
                    nc.tensor.matmul(
                        psum[:, :], ones[:, :], hti[:, 1:L + 1],
                        start=(i == 0), stop=(i == DT - 1),
                    )
                    nc.tensor.matmul(
                        psq[:, :], ones[:, :], sqs[i][:, :],
                        start=(i == 0), stop=(i == DT - 1),
                    )
                    if i == DT - 1:
                        # emit the scalar chain right away: frees the stats
                        # PSUM banks quickly and finishes early on DVE/ACT
                        mu = stats.tile([128, L], F32, tag="mu", bufs=2, name=f"mu_{s}_{ln}")
                        nc.scalar.copy(out=mu, in_=psum)
                        musq = stats.tile([128, L], F32, tag="musq", bufs=2, name=f"musq_{s}_{ln}")
                        nc.vector.tensor_mul(musq, mu, mu)
                        var = stats.tile([128, L], F32, tag="var", bufs=2, name=f"var_{s}_{ln}")
                        nc.vector.tensor_sub(var, psq, musq)
                        std = stats.tile([128, L], F32, tag="std", bufs=2, name=f"std_{s}_{ln}")
                        nc.scalar.activation(
                            out=std, in_=var, func=AF.Sqrt, bias=eps_sb[:, 0:1], scale=1.0
                        )
                        rstd = stats.tile([128, L], F32, tag="rstd", bufs=2, name=f"rstd_{s}_{ln}")
                        nc.vector.reciprocal(out=rstd, in_=std)
                        done["mu"] = mu
                        done["rstd"] = rstd

                def finish():
                    return done["mu"], done["rstd"]

                return stats_cb, finish

            def emit_ln_apply(s, ln, hin, mu, rstd, tag):
                hout = [
                    acts.tile([128, L + 2], F32R, name=f"{tag}ln{ln}_{i}_{s}", tag=f"{tag}_{i}", bufs=3) for i in range(DT)
                ]
                for i in range(DT):
                    t = acts.tile([128, L], F32, tag="lnt", bufs=3, name=f"lnt_{s}_{ln}_{i}")
                    nc.vector.tensor_sub(t, hin[i][:, 1:L + 1].bitcast(F32), mu)
                    nc.vector.tensor_mul(t, t, rstd)
                    nc.vector.memset(hout[i][:, 0:1].bitcast(F32), 0.0)
                    nc.vector.memset(hout[i][:, L + 1:L + 2].bitcast(F32), 0.0)
                    nc.scalar.activation(
                        out=hout[i][:, 1:L + 1],
                        in_=t,
                        func=AF.Identity,
                        bias=lnb_sb[:, ln, i:i + 1],
                        scale=lng_sb[:, ln, i:i + 1],
                    )
                return hout

            def emit_linear(s, h6):
                pl = ppool.tile([1, L], F32, tag="psq", bufs=2, space="PSUM", name=f"pl_{s}")
                for i in range(DT):
                    nc.tensor.matmul(
                        pl[:, :],
                        linw_sb[:, i:i + 1],
                        h6[i][:, 1:L + 1],
                        start=(i == 0),
                        stop=(i == DT - 1),
                    )
                dur_sb = io.tile([1, L], F32, tag="dur", name=f"dur_{s}")
                nc.scalar.activation(
                    out=dur_sb, in_=pl, func=AF.Identity, bias=linb_sb[0:1, 0:1],
                    scale=1.0,
                )
                nc.sync.dma_start(out=durd[s:s + 1, :], in_=dur_sb)

            # ---- software-pipelined emission over samples ----
            stA = {}   # s -> (finishLN1, h2)
            stB = {}   # s -> (finishLN2, h5)

            def phase_A(s, h0=None):
                if h0 is None:
                    h0 = emit_transpose(s)
                h1 = conv_layer(s, 0, h0, "ha")
                cb1, fin1 = make_stats(s, 0)
                h2 = conv_layer(s, 1, h1, "hb", stats_cb=cb1)
                stA[s] = (fin1, h2)

            def phase_B(s):
                emit_gather(s)
                fin1, h2 = stA.pop(s)
                mu1, rstd1 = fin1()
                h3 = emit_ln_apply(s, 0, h2, mu1, rstd1, "hc")
                h4 = conv_layer(s, 2, h3, "ha")
                cb2, fin2 = make_stats(s, 1)
                h5 = conv_layer(s, 3, h4, "hb", stats_cb=cb2)
                stB[s] = (fin2, h5)

            stC = {}

            def phase_Cchain(s):
                fin2, h5 = stB.pop(s)
                mu2, rstd2 = fin2()
                h6 = emit_ln_apply(s, 1, h5, mu2, rstd2, "hd")
                stC[s] = h6

            def phase_Clin(s):
                emit_linear(s, stC.pop(s))

            h0_first = emit_transpose(0)
            nc.sync.dma_start(out=cvb_sb, in_=cvb.rearrange("l (i p) -> p l i", p=128))
            phase_A(0, h0_first)
            emit_param_loads()
            phase_A(1)
            phase_B(0)
            phase_A(2)
            phase_B(1)
            phase_Cchain(0)
            phase_A(3)
            phase_B(2)
            phase_Cchain(1)
            phase_Clin(0)
            phase_B(3)
            phase_Cchain(2)
            phase_Clin(1)
            phase_Cchain(3)
            phase_Clin(2)
            phase_Clin(3)

    nc.compile()
    return nc


_prog_cache = {}


def _get_program(T_p):
    if T_p not in _prog_cache:
        _prog_cache[T_p] = _build_program(T_p)
    return _prog_cache[T_p]


def kernel(x, durations, c1a_w, c1a_b, c1b_w, c1b_b, ln1_g, ln1_b,
           c2a_w, c2a_b, c2b_w, c2b_b, ln2_g, ln2_b, lin_w, lin_b):
    x = np.ascontiguousarray(np.asarray(x, dtype=np.float32))
    durations = np.asarray(durations, dtype=np.int32)

    # ---- host-side index computation (mirrors reference LR exactly) ----
    cum = np.cumsum(durations, axis=1)              # [B, L]
    total = cum[:, -1]                              # [B]
    T = int(total.max())
    T_p = max(128, -(-T // 128) * 128)
    C = T_p // 128

    t_ar = np.arange(T_p)
    ZROW = BS * L                                    # zero row index in xz
    # per sample: frame t -> row (s*L + searchsorted(cum, t, 'right')) or ZROW
    idx_all = np.empty((B, T_p), dtype=np.int16)
    for b in range(B):
        ii = np.searchsorted(cum[b], t_ar, side="right")
        valid = t_ar < total[b]
        loc = np.where(valid, (b % BS) * L + np.minimum(ii, L - 1), ZROW)
        idx_all[b] = loc.astype(np.int16)

    # ---- pack per-core inputs ----
    # weights, pre-transposed to [d_in, d_out] blocks, layer-major
    def pack_w(w):
        # w: [D_out, D_in, 3] -> [128, 27*128]; block b = o*9 + i*3 + tap
        out = np.empty((128, 27 * 128), dtype=np.float32)
        for o in range(DT):
            for i in range(DT):
                for tap in range(3):
                    blk = o * 9 + i * 3 + tap
                    out[:, blk * 128:(blk + 1) * 128] = (
                        w[o * 128:(o + 1) * 128, i * 128:(i + 1) * 128, tap].T
                    )
        return out

    wts = np.stack([pack_w(np.asarray(w, np.float32))
                    for w in (c1a_w, c1b_w, c2a_w, c2b_w)])   # [4,128,3456]
    cvb = np.stack([np.asarray(v, np.float32)
                    for v in (c1a_b, c1b_b, c2a_b, c2b_b)])   # [4,384]
    lng = np.stack([np.asarray(ln1_g, np.float32), np.asarray(ln2_g, np.float32)])
    lnb = np.stack([np.asarray(ln1_b, np.float32), np.asarray(ln2_b, np.float32)])
    linw = np.asarray(lin_w, np.float32).reshape(D)
    linb = np.asarray(lin_b, np.float32).reshape(1, 1)

    in_maps = []
    for core in range(NCORES):
        sl = slice(core * BS, (core + 1) * BS)
        xs = x[sl].reshape(BS * L, D)
        xzc = np.concatenate([xs, np.zeros((1, D), np.float32)], axis=0)
        idxc = idx_all[sl]                              # [BS, T_p]
        # wrap: element j -> [j % 16, j // 16]
        idxw = np.ascontiguousarray(
            idxc.reshape(BS, T_p // 16, 16).transpose(0, 2, 1)
        )                                               # [BS, 16, T_p//16]
        idxw = np.ascontiguousarray(np.tile(idxw, (1, 8, 1)))  # [BS, 128, .]
        in_maps.append({
            "xz": xzc, "wts": wts, "cvb": cvb, "lng": lng, "lnb": lnb,
            "linw": linw, "linb": linb, "idx": idxw,
        })

    nc = _get_program(T_p)
    res = run_bass_kernel_spmd(nc, in_maps, list(range(NCORES)))

    out_full = np.empty((B, T, D), dtype=np.float32)
    dur_full = np.empty((B, L), dtype=np.float32)
    for core in range(NCORES):
        r = res.results[core]
        out_full[core * BS:(core + 1) * BS] = r["out"][:, :T, :]
        dur_full[core * BS:(core + 1) * BS] = r["dur"]
    return (out_full, dur_full)


# revision 28
# speedup vs baseline: 1.0919x; 1.0109x over previous
"""Trainium2 Bass kernel for nn_Length_Regulator (DurationPredictor + LR expansion).

Sharding: pure data-parallel over batch (32 samples -> 4 per core x 8 cores),
params replicated. Per-core program:
  - DurationPredictor: 4 conv1d(k=3, SAME) layers as PE matmuls (float32r,
    D-major layout), 2 LayerNorms (stats via ones-matmul over partitions),
    final linear head -> dur_preds.
  - LR expansion: host computes searchsorted frame->phoneme indices (the
    reference itself computes T host-side); device gathers rows with
    dma_gather (SWDGE) from a [4*512+1, 384] table whose last row is zero
    (used for padding frames), stores [T_p, 384] per sample. Runs on the
    DMA engines fully overlapped with the conv matmuls on PE.
"""

import sys

for _p in ("/opt/trn_rl_repo", "/root/.axon_site/_ro/trn_rl_repo"):
    if _p not in sys.path:
        sys.path.append(_p)

import numpy as np

import concourse.bass as bass
import concourse.bacc as bacc
import concourse.mybir as mybir
import concourse.tile as tile
import bass_rust
from concourse.vector_clock import ScopedClock
from concourse.bass_utils import run_bass_kernel_spmd
from concourse.masks import make_identity
from concourse import library_config

F32 = mybir.dt.float32
F32R = mybir.dt.float32r
I16 = mybir.dt.int16
AF = mybir.ActivationFunctionType

B, L, D = 32, 512, 384
NCORES = 8
BS = B // NCORES          # samples per core
DT = D // 128             # d tiles (3)
LN_EPS = 1e-5
GMAX = 5                  # gather chunk: columns of 128 frames per dma_gather

# ---------------------------------------------------------------------------
# Tile framework patch: this container's walrus rejects >1 sem-wait on the
# kernel-tail InstDrain ("Too many sync wait commands"). Split the global-clock
# waits onto one nop per wait instead.
_PATCHED = False


def _patch_tile_drain():
    global _PATCHED
    if _PATCHED:
        return
    _PATCHED = True

    def _drain_and_barrier(self, tick_clock, wait_clock):
        drain_inst = self.nc.sync.drain()
        wait_clock.add_sem_waits(
            drain_inst.ins, ScopedClock({None: tick_clock.global_clock})
        )
        si = drain_inst.ins.sync_info
        waits = list(si.on_wait) if si is not None else []
        if len(waits) > 1:
            si.on_wait = waits[:1]
            for k in range(1, len(waits)):
                extra = self.nc.sync.nop(nofuse=True, hint="tile_drain_wait_split")
                extra.ins.sync_info = bass_rust.SyncInfo(
                    on_wait=[waits[k]], on_update=[]
                )
        self.nc.all_engine_barrier()
        popped = self.nc._tile_sem_poison_stack.pop()
        assert popped is self._sem_poison
        self.nc.clear_and_free_semaphores(list(self.sems.allocated().values()))
        self.nc.all_engine_barrier()

    tile.TileContext._drain_and_barrier = _drain_and_barrier


# ---------------------------------------------------------------------------


def _build_program(T_p):
    """One SPMD program; every core runs it on its own 4-sample shard."""
    _patch_tile_drain()
    C = T_p // 128            # gather columns per sample
    CH = C // 16 * 2          # int16 idx cols per sample... (see idx packing)
    IDXC = T_p // 16          # wrapped idx columns per sample

    nc = bacc.Bacc("TRN2", target_bir_lowering=False, debug=False, num_devices=NCORES)
    xz = nc.declare_dram_parameter("xz", [BS * L + 1, D], F32, isOutput=False)
    wts = nc.declare_dram_parameter("wts", [4, 128, 27 * 128], F32, isOutput=False)
    cvb = nc.declare_dram_parameter("cvb", [4, D], F32, isOutput=False)
    lng = nc.declare_dram_parameter("lng", [2, D], F32, isOutput=False)
    lnb = nc.declare_dram_parameter("lnb", [2, D], F32, isOutput=False)
    linw = nc.declare_dram_parameter("linw", [D], F32, isOutput=False)
    linb = nc.declare_dram_parameter("linb", [1, 1], F32, isOutput=False)
    idxd = nc.declare_dram_parameter("idx", [BS, 128, IDXC], I16, isOutput=False)
    outd = nc.declare_dram_parameter("out", [BS, T_p, D], F32, isOutput=True)
    durd = nc.declare_dram_parameter("dur", [BS, L], F32, isOutput=True)

    with tile.TileContext(nc) as tc:
        with (
            tc.tile_pool(name="const", bufs=1) as const,
            tc.tile_pool(name="wpool", bufs=1) as wpool,
            tc.tile_pool(name="acts", bufs=3) as acts,
            tc.tile_pool(name="stats", bufs=2) as stats,
            tc.tile_pool(name="gather", bufs=4) as gather,
            tc.tile_pool(name="io", bufs=4) as io,
            tc.tile_pool(name="ppool", bufs=1, space="PSUM") as ppool,
        ):
            # ---- constants (weights first: conv1a blocks on layer 0) ----
            nc.gpsimd.load_library(library_config.mlp)
            ident = const.tile([128, 128], F32)
            make_identity(nc, ident)
            ones_f = const.tile([128, 128], F32)
            nc.gpsimd.memset(ones_f, 1.0 / D)
            ones = const.tile([128, 128], F32R)
            nc.vector.tensor_copy(ones, ones_f)
            eps_sb = const.tile([128, 1], F32)
            nc.gpsimd.memset(eps_sb, LN_EPS)
            w_sb = [[const.tile([128, 9 * 128], F32R, name=f"w_sb_{_l}_{_o}")
                     for _o in range(DT)] for _l in range(4)]
            for _l in range(4):
                for _o in range(DT):
                    nc.gpsimd.dma_start(
                        out=w_sb[_l][_o], in_=wts[_l, :, _o * 9 * 128:(_o + 1) * 9 * 128]
                    )
            cvb_sb = const.tile([128, 4, DT], F32)
            lng_sb = const.tile([128, 2, DT], F32)
            lnb_sb = const.tile([128, 2, DT], F32)
            linw_sb = const.tile([128, DT], F32R)
            linb_sb = const.tile([1, 1], F32)
            idx_sb = const.tile([128, BS, IDXC], I16)

            wgb_r = const.tile([128, 2, DT], F32R)
            swgb_sb = const.tile([1, 2], F32)

            def emit_param_loads():
                nc.sync.dma_start(out=lng_sb, in_=lng.rearrange("l (i p) -> p l i", p=128))
                nc.sync.dma_start(out=lnb_sb, in_=lnb.rearrange("l (i p) -> p l i", p=128))
                nc.gpsimd.dma_start(out=linw_sb, in_=linw.rearrange("(i p) -> p i", p=128))
                nc.sync.dma_start(out=linb_sb, in_=linb[:, :])
                nc.sync.dma_start(out=idx_sb, in_=idxd.rearrange("s p c -> p s c"))
                # fold LN2's affine into the linear head: wg = w*g, wb = w*b
                wgb_f = const.tile([128, 2, DT], F32)
                nc.vector.tensor_mul(wgb_f[:, 0, :], linw_sb.bitcast(F32), lng_sb[:, 1, :])
                nc.vector.tensor_mul(wgb_f[:, 1, :], linw_sb.bitcast(F32), lnb_sb[:, 1, :])
                nc.vector.tensor_copy(wgb_r, wgb_f)
                psw = ppool.tile([1, 2], F32, tag="pstat", bufs=1, space="PSUM", name="psw")
                for i in range(DT):
                    nc.tensor.matmul(
                        psw[:, :], ones[:, 0:1], wgb_r[:, :, i],
                        start=(i == 0), stop=(i == DT - 1),
                    )
                # ones carry 1/D; undo it here
                nc.scalar.mul(out=swgb_sb, in_=psw, mul=float(D))

            def w_tile(layer, o, i, tap):
                blk = i * 3 + tap
                return w_sb[layer][o][:, blk * 128:(blk + 1) * 128]

            def emit_gather(s):
                c0 = 0
                while c0 < C:
                    g = min(GMAX, C - c0)
                    gt = gather.tile([128, GMAX, D], F32, tag="gt", name=f"gt_{s}_{c0}")
                    nc.gpsimd.dma_gather(
                        out_ap=gt[:, :g, :],
                        in_ap=xz[:, :],
                        idxs_ap=idx_sb[:, s, c0 * 8:(c0 + g) * 8],
                        num_idxs=128 * g,
                        num_idxs_reg=128 * g,
                        elem_size=D,
                    )
                    dst = outd[s, c0 * 128:(c0 + g) * 128, :].rearrange(
                        "(c p) d -> p c d", p=128
                    )
                    nc.sync.dma_start(out=dst, in_=gt[:, :g, :])
                    c0 += g

            def emit_transpose(s):
                h0 = [acts.tile([128, L + 2], F32R, name=f"h0_{i}_{s}", tag=f"h0_{i}", bufs=2) for i in range(DT)]
                pxt = [
                    ppool.tile([128, L], F32, name=f"pxt_{i}_{s}", tag=f"pxt_{i}", bufs=1, space="PSUM")
                    for i in range(DT)
                ]
                for c in range(4):
                    xn = io.tile([128, D], F32, tag="xn", name=f"xn_{s}_{c}")
                    nc.sync.dma_start(
                        out=xn, in_=xz[s * L + c * 128:s * L + (c + 1) * 128, :]
                    )
                    for i in range(DT):
                        nc.tensor.transpose(
                            pxt[i][:, c * 128:(c + 1) * 128],
                            xn[:, i * 128:(i + 1) * 128],
                            ident,
                        )
                for i in range(DT):
                    nc.vector.memset(h0[i][:, 0:1].bitcast(F32), 0.0)
                    nc.vector.memset(h0[i][:, L + 1:L + 2].bitcast(F32), 0.0)
                    nc.vector.tensor_copy(h0[i][:, 1:L + 1], pxt[i])
                return h0

            def conv_layer(s, layer, hin, tag, stats_cb=None):
                # stats_cb(i, hout_i): emit per-tile stats work lagged by one chunk
                hout = [
                    acts.tile([128, L + 2], F32R, name=f"{tag}{layer}_{i}_{s}", tag=f"{tag}_{i}", bufs=2) for i in range(DT)
                ]
                for o in range(DT):
                    pc = ppool.tile([128, L], F32, tag="pconv", bufs=2, space="PSUM", name=f"pc_{s}_{layer}_{o}")
                    first = True
                    for tap in (1, 0, 2):
                        for i in range(DT):
                            nc.tensor.matmul(
                                pc[:, :],
                                w_tile(layer, o, i, tap),
                                hin[i][:, tap:tap + L],
                                start=first,
                                stop=(tap == 2 and i == DT - 1),
                            )
                            first = False
                    nc.vector.memset(hout[o][:, 0:1].bitcast(F32), 0.0)
                    nc.vector.memset(hout[o][:, L + 1:L + 2].bitcast(F32), 0.0)
                    nc.scalar.activation(
                        out=hout[o][:, 1:L + 1],
                        in_=pc,
                        func=AF.Relu,
                        bias=cvb_sb[:, layer, o:o + 1],
                        scale=1.0,
                    )
                    if stats_cb is not None and o >= 1:
                        stats_cb(o - 1, hout[o - 1])
                if stats_cb is not None:
                    stats_cb(DT - 1, hout[DT - 1])
                return hout

            def make_stats(s, ln):
                """Returns (stats_cb, finish) where stats_cb emits per-tile sum/
                sumsq matmuls and finish() emits the scalar chain -> (mu, rstd)."""
                psum = ppool.tile([128, L], F32, tag="pstat", bufs=1, space="PSUM", name=f"pstat_{s}_{ln}")
                psq = ppool.tile([128, L], F32, tag="psq", bufs=2, space="PSUM", name=f"psq_{s}_{ln}")
                sqs = [acts.tile([128, L], F32R, name=f"sq_{i}_{s}_{ln}", tag="sq", bufs=4) for i in range(DT)]

                done = {}

                def stats_cb(i, hti):
                    nc.vector.tensor_mul(sqs[i], hti[:, 1:L + 1].bitcast(F32), hti[:, 1:L + 1].bitcast(F32))
                    nc.tensor.matmul(
                        psum[:, :], ones[:, :], hti[:, 1:L + 1],
                        start=(i == 0), stop=(i == DT - 1),
                    )
                    nc.tensor.matmul(
                        psq[:, :], ones[:, :], sqs[i][:, :],
                        start=(i == 0), stop=(i == DT - 1),
                    )
                    if i == DT - 1:
                        # emit the scalar chain right away: frees the stats
                        # PSUM banks quickly and finishes early on DVE/ACT
                        mu = stats.tile([128, L], F32, tag="mu", bufs=2, name=f"mu_{s}_{ln}")
                        nc.scalar.copy(out=mu, in_=psum)
                        musq = stats.tile([128, L], F32, tag="musq", bufs=2, name=f"musq_{s}_{ln}")
                        nc.vector.tensor_mul(musq, mu, mu)
                        var = stats.tile([128, L], F32, tag="var", bufs=2, name=f"var_{s}_{ln}")
                        nc.vector.tensor_sub(var, psq, musq)
                        std = stats.tile([128, L], F32, tag="std", bufs=2, name=f"std_{s}_{ln}")
                        nc.scalar.activation(
                            out=std, in_=var, func=AF.Sqrt, bias=eps_sb[:, 0:1], scale=1.0
                        )
                        rstd = stats.tile([128, L], F32, tag="rstd", bufs=2, name=f"rstd_{s}_{ln}")
                        nc.vector.reciprocal(out=rstd, in_=std)
                        done["mu"] = mu
                        done["rstd"] = rstd

                def finish():
                    return done["mu"], done["rstd"]

                return stats_cb, finish

            def emit_ln_apply(s, ln, hin, mu, rstd, tag):
                hout = [
                    acts.tile([128, L + 2], F32R, name=f"{tag}ln{ln}_{i}_{s}", tag=f"{tag}_{i}", bufs=3) for i in range(DT)
                ]
                for i in range(DT):
                    t = acts.tile([128, L], F32, tag="lnt", bufs=3, name=f"lnt_{s}_{ln}_{i}")
                    nc.vector.tensor_sub(t, hin[i][:, 1:L + 1].bitcast(F32), mu)
                    nc.vector.tensor_mul(t, t, rstd)
                    nc.vector.memset(hout[i][:, 0:1].bitcast(F32), 0.0)
                    nc.vector.memset(hout[i][:, L + 1:L + 2].bitcast(F32), 0.0)
                    nc.scalar.activation(
                        out=hout[i][:, 1:L + 1],
                        in_=t,
                        func=AF.Identity,
                        bias=lnb_sb[:, ln, i:i + 1],
                        scale=lng_sb[:, ln, i:i + 1],
                    )
                return hout

            def emit_linear(s, h6):
                pl = ppool.tile([1, L], F32, tag="psq", bufs=2, space="PSUM", name=f"pl_{s}")
                for i in range(DT):
                    nc.tensor.matmul(
                        pl[:, :],
                        linw_sb[:, i:i + 1],
                        h6[i][:, 1:L + 1],
                        start=(i == 0),
                        stop=(i == DT - 1),
                    )
                dur_sb = io.tile([1, L], F32, tag="dur", name=f"dur_{s}")
                nc.scalar.activation(
                    out=dur_sb, in_=pl, func=AF.Identity, bias=linb_sb[0:1, 0:1],
                    scale=1.0,
                )
                nc.sync.dma_start(out=durd[s:s + 1, :], in_=dur_sb)

            # ---- software-pipelined emission over samples ----
            stA = {}   # s -> (finishLN1, h2)
            stB = {}   # s -> (finishLN2, h5)

            def phase_A(s, h0=None):
                if h0 is None:
                    h0 = emit_transpose(s)
                h1 = conv_layer(s, 0, h0, "ha")
                cb1, fin1 = make_stats(s, 0)
                h2 = conv_layer(s, 1, h1, "hb", stats_cb=cb1)
                stA[s] = (fin1, h2)

            def phase_B(s):
                emit_gather(s)
                fin1, h2 = stA.pop(s)
                mu1, rstd1 = fin1()
                h3 = emit_ln_apply(s, 0, h2, mu1, rstd1, "hc")
                h4 = conv_layer(s, 2, h3, "ha")
                cb2, fin2 = make_stats(s, 1)
                h5 = conv_layer(s, 3, h4, "hb", stats_cb=cb2)
                # A(t) = sum_d (w*g)_d h5[d, t] -- depends only on h5
                pA = ppool.tile([1, L], F32, tag="psq", bufs=2, space="PSUM", name=f"pA_{s}")
                for i in range(DT):
                    nc.tensor.matmul(
                        pA[:, :], wgb_r[:, 0:1, i], h5[i][:, 1:L + 1],
                        start=(i == 0), stop=(i == DT - 1),
                    )
                stB[s] = (fin2, pA)

            def phase_C(s):
                fin2, pA = stB.pop(s)
                mu2, rstd2 = fin2()
                t1 = stats.tile([1, L], F32, tag="t1", bufs=2, name=f"t1_{s}")
                nc.vector.tensor_scalar_mul(
                    out=t1, in0=mu2[0:1, :], scalar1=swgb_sb[0:1, 0:1]
                )
                t2 = stats.tile([1, L], F32, tag="t2", bufs=2, name=f"t2_{s}")
                nc.vector.tensor_sub(t2, pA[0:1, :], t1)
                t3 = stats.tile([1, L], F32, tag="t3", bufs=2, name=f"t3_{s}")
                nc.vector.tensor_mul(t3, t2, rstd2[0:1, :])
                dur_sb = io.tile([1, L], F32, tag="dur", name=f"dur_{s}")
                nc.vector.tensor_scalar(
                    out=dur_sb, in0=t3,
                    scalar1=swgb_sb[0:1, 1:2], scalar2=linb_sb[0:1, 0:1],
                    op0=mybir.AluOpType.add, op1=mybir.AluOpType.add,
                )
                nc.sync.dma_start(out=durd[s:s + 1, :], in_=dur_sb)

            h0_first = emit_transpose(0)
            nc.sync.dma_start(out=cvb_sb, in_=cvb.rearrange("l (i p) -> p l i", p=128))
            phase_A(0, h0_first)
            emit_param_loads()
            phase_A(1)
            phase_B(0)
            phase_A(2)
            phase_B(1)
            phase_C(0)
            phase_A(3)
            phase_B(2)
            phase_C(1)
            phase_B(3)
            phase_C(2)
            phase_C(3)

    nc.compile()
    return nc


_prog_cache = {}


def _get_program(T_p):
    if T_p not in _prog_cache:
        _prog_cache[T_p] = _build_program(T_p)
    return _prog_cache[T_p]


def kernel(x, durations, c1a_w, c1a_b, c1b_w, c1b_b, ln1_g, ln1_b,
           c2a_w, c2a_b, c2b_w, c2b_b, ln2_g, ln2_b, lin_w, lin_b):
    x = np.ascontiguousarray(np.asarray(x, dtype=np.float32))
    durations = np.asarray(durations, dtype=np.int32)

    # ---- host-side index computation (mirrors reference LR exactly) ----
    cum = np.cumsum(durations, axis=1)              # [B, L]
    total = cum[:, -1]                              # [B]
    T = int(total.max())
    T_p = max(128, -(-T // 128) * 128)
    C = T_p // 128

    t_ar = np.arange(T_p)
    ZROW = BS * L                                    # zero row index in xz
    # per sample: frame t -> row (s*L + searchsorted(cum, t, 'right')) or ZROW
    idx_all = np.empty((B, T_p), dtype=np.int16)
    for b in range(B):
        ii = np.searchsorted(cum[b], t_ar, side="right")
        valid = t_ar < total[b]
        loc = np.where(valid, (b % BS) * L + np.minimum(ii, L - 1), ZROW)
        idx_all[b] = loc.astype(np.int16)

    # ---- pack per-core inputs ----
    # weights, pre-transposed to [d_in, d_out] blocks, layer-major
    def pack_w(w):
        # w: [D_out, D_in, 3] -> [128, 27*128]; block b = o*9 + i*3 + tap
        out = np.empty((128, 27 * 128), dtype=np.float32)
        for o in range(DT):
            for i in range(DT):
                for tap in range(3):
                    blk = o * 9 + i * 3 + tap
                    out[:, blk * 128:(blk + 1) * 128] = (
                        w[o * 128:(o + 1) * 128, i * 128:(i + 1) * 128, tap].T
                    )
        return out

    wts = np.stack([pack_w(np.asarray(w, np.float32))
                    for w in (c1a_w, c1b_w, c2a_w, c2b_w)])   # [4,128,3456]
    cvb = np.stack([np.asarray(v, np.float32)
                    for v in (c1a_b, c1b_b, c2a_b, c2b_b)])   # [4,384]
    lng = np.stack([np.asarray(ln1_g, np.float32), np.asarray(ln2_g, np.float32)])
    lnb = np.stack([np.asarray(ln1_b, np.float32), np.asarray(ln2_b, np.float32)])
    linw = np.asarray(lin_w, np.float32).reshape(D)
    linb = np.asarray(lin_b, np.float32).reshape(1, 1)

    in_maps = []
    for core in range(NCORES):
        sl = slice(core * BS, (core + 1) * BS)
        xs = x[sl].reshape(BS * L, D)
        xzc = np.concatenate([xs, np.zeros((1, D), np.float32)], axis=0)
        idxc = idx_all[sl]                              # [BS, T_p]
        # wrap: element j -> [j % 16, j // 16]
        idxw = np.ascontiguousarray(
            idxc.reshape(BS, T_p // 16, 16).transpose(0, 2, 1)
        )                                               # [BS, 16, T_p//16]
        idxw = np.ascontiguousarray(np.tile(idxw, (1, 8, 1)))  # [BS, 128, .]
        in_maps.append({
            "xz": xzc, "wts": wts, "cvb": cvb, "lng": lng, "lnb": lnb,
            "linw": linw, "linb": linb, "idx": idxw,
        })

    nc = _get_program(T_p)
    res = run_bass_kernel_spmd(nc, in_maps, list(range(NCORES)))

    out_full = np.empty((B, T, D), dtype=np.float32)
    dur_full = np.empty((B, L), dtype=np.float32)
    for core in range(NCORES):
        r = res.results[core]
        out_full[core * BS:(core + 1) * BS] = r["out"][:, :T, :]
        dur_full[core * BS:(core + 1) * BS] = r["dur"]
    return (out_full, dur_full)
